# revision 40
# baseline (speedup 1.0000x reference)
"""LogSparse attention kernel for 8 TRN2 NeuronCores.

Problem: B=4, S=2048, H=1024, 16 heads x 64 dim. Logsparse mask: query i
attends key j iff i-j == 0 or i-j == 2^k (so <=12 keys per query, at
power-of-2 offsets).

Sharding: core c -> batch b = c//2, head-group g = c%2 (8 heads each).
Each core computes q/k/v projections for its (batch, head-group) and the
sparse attention, writing out[b, :, g*512:(g+1)*512].

Device algorithm (per core), v2 (interleaved):
  - DMA-transpose X (bf16) -> XT [h, s] on SBUF in 4 s-chunks so the
    QK projections start as chunks land.
  - QT/KT = W @ XT ([dh, s], dh on partitions), V = X @ WvT ([s, dh]).
  - Attention is key-block-major: key block kb serves query blocks
    qb' in {kb, kb+1} (dense logsparse band, one N=256 matmul) and
    qb' in {kb+2, kb+4, kb+8} (single surviving diagonal each, N=128),
    all sharing the KT[kb] stationary. Scores land in one psum strip
    [128 kj, <=640 qi]; one ACT exp(0.125*s) -> bf16 and one DVE
    multiplicative-mask produce PT. PV (lhsT=PT slice, rhs=V[kb]) and
    row-sum (rhs=ones col) accumulate into per-qb psum; finalize reads
    psum directly (DVE reciprocal + tensor_scalar_mul -> bf16 outs).
  - The attention phase for head h is SOFTWARE-PIPELINED (PV lags
    scores by D=2 key blocks) and INTERLEAVED with the remaining
    projection matmul chains (V during head 0, QK m-tiles 1..3 paced
    through heads 1..5) so the scalar/vector engines' exp/mask work
    hides under projection matmuls instead of pacing the tensor queue.
  - The logsparse mask pattern is shift-invariant across key blocks, so
    when the additive attention_mask is all-zero (the common case) a
    single [128, 640] mask tile is loaded and shared by every kb.
Softmax max-subtraction is skipped: scores*0.125 has std ~0.4 for this
problem family, far from exp overflow. Output is written bf16 (inputs
are unit-scale; bf16 rounding is ~0.4% vs the 2e-2 tolerance) and
upcast to f32 on host.
"""

import numpy as np
import ml_dtypes

import concourse.bass as bass
from concourse import bacc
import concourse.mybir as mybir
from concourse.tile import TileContext
from concourse.bass_utils import run_bass_kernel_spmd

B, S, H = 4, 2048, 1024
NH, HD = 16, 64
G = 2  # head groups per batch
HPC = NH // G  # heads per core = 8
GD = HPC * HD  # 512 group dim
NQB = S // 128  # 16 query blocks
KCH = H // 128  # 8 contraction chunks

BF16 = mybir.dt.bfloat16
F32 = mybir.dt.float32
NPBF16 = ml_dtypes.bfloat16

FAR = (2, 4, 8)
DELAY = 2  # PV lags scores by this many key blocks


def _allowed(diff):
    return (diff == 0) | ((diff > 0) & ((diff & (diff - 1)) == 0))


def _consumers(kb):
    """Query blocks served by key block kb, as (qb, col offset in the
    psum strip). Dense pair first (contiguous N), then far diagonals.
    Valid consumers always form a contiguous col prefix."""
    out = [(kb, 0)]
    if kb + 1 < NQB:
        out.append((kb + 1, 128))
    for i, d in enumerate(FAR):
        if kb + d < NQB:
            out.append((kb + d, 256 + 128 * i))
    return out


def build_program(has_bias: bool, has_am: bool):
    nc = bacc.Bacc("TRN2", target_bir_lowering=False)

    # x arrives pre-transposed from the host ([h%128, h//128, s]) so the
    # load is plain big-packet DMA instead of the slow 256B-packet xbar
    # transpose. Wq/Wk are m-major so each 128-wide dh tile is one small
    # contiguous load (the m=0 slices gate the critical path).
    x_d = nc.declare_dram_parameter("x", [128, 8, KCH, 256], BF16, isOutput=False)
    wq_d = nc.declare_dram_parameter("wq", [128, 4, KCH, 128], BF16, isOutput=False)
    wk_d = nc.declare_dram_parameter("wk", [128, 4, KCH, 128], BF16, isOutput=False)
    wv_d = nc.declare_dram_parameter("wv", [128, KCH, GD], BF16, isOutput=False)
    # multiplicative masks: shift-invariant [pj, 640] strip unless a
    # nonzero additive attention mask forces a per-kb version.
    if has_am:
        masks_d = nc.declare_dram_parameter(
            "masks", [128, NQB, 640], BF16, isOutput=False
        )
    else:
        masks_d = nc.declare_dram_parameter(
            "masks", [128, 2, 640], BF16, isOutput=False
        )
    if has_bias:
        bqm_d = nc.declare_dram_parameter("bqm", [1, 4, 128], BF16, isOutput=False)
        bkm_d = nc.declare_dram_parameter("bkm", [1, 4, 128], BF16, isOutput=False)
        bv_d = nc.declare_dram_parameter("bv", [1, GD], BF16, isOutput=False)
        ones_row_d = nc.declare_dram_parameter(
            "ones_row", [1, 512], BF16, isOutput=False
        )
    out_d = nc.declare_dram_parameter("out", [S, GD], BF16, isOutput=True)

    with TileContext(nc) as tc:
        with (
            tc.tile_pool(name="const", bufs=1) as const_pool,
            tc.tile_pool(name="big", bufs=1) as big_pool,
        ):
            # ---- resident SBUF tensors ----
            # chunk-major so each 512KB s-chunk is one contiguous DMA with
            # 4KB-per-partition descriptors on both the DRAM and SBUF side
            xt = big_pool.tile([128, 8, KCH, 256], BF16, tag="xt")
            wq = big_pool.tile([128, 4, KCH, 128], BF16, tag="wq")
            wk = big_pool.tile([128, 4, KCH, 128], BF16, tag="wk")
            wv = big_pool.tile([128, KCH, GD], BF16, tag="wv")
            qt = big_pool.tile([128, 4, S], BF16, tag="qt")  # [dh%128, m, s]
            kt = big_pool.tile([128, 4, S], BF16, tag="kt")
            vv = big_pool.tile([128, NQB, HPC, HD + 1], BF16, tag="v")
            if has_am:
                masks = const_pool.tile([128, NQB, 640], BF16, tag="masks")
            else:
                masks = const_pool.tile([128, 2, 640], BF16, tag="masks")
            outs = big_pool.tile([128, NQB, GD], BF16, tag="outs")

            # ---- loads on 3 parallel DMA channels (~100 GB/s each):
            # SP HWDGE: x s-chunks 0,1; ACT HWDGE: x s-chunks 2,3;
            # GpSimd SWDGE: m-sliced Wq/Wk (m=0 first, gating the first
            # projections), then Wv + masks + remaining m-slices. ----
            # scratch for PE warmup: memset on gpsimd BEFORE the DMA
            # issues so the first warmup matmul fires ~3us in and the
            # PE clock ramps during the load wait
            scratch = const_pool.tile([128, 512], BF16, tag="warm")
            nc.gpsimd.memset(scratch[:], 0.0)
            for ch in range(4):
                nc.sync.dma_start(xt[:, ch], x_d[:, ch])
            for ch in range(4, 8):
                nc.scalar.dma_start(xt[:, ch], x_d[:, ch])
            nc.gpsimd.dma_start(wq[:, 0], wq_d[:, 0])
            nc.gpsimd.dma_start(wk[:, 0], wk_d[:, 0])
            nc.gpsimd.dma_start(wv[:], wv_d[:])
            nc.gpsimd.dma_start(masks[:], masks_d[:])
            for m in (1, 2, 3):
                nc.gpsimd.dma_start(wq[:, m], wq_d[:, m])
                nc.gpsimd.dma_start(wk[:, m], wk_d[:, m])
            nc.vector.memset(vv[:, :, :, HD : HD + 1], 1.0)
            if has_bias:
                bqm = const_pool.tile([1, 4, 128], BF16, tag="bqm")
                bkm = const_pool.tile([1, 4, 128], BF16, tag="bkm")
                bvr = const_pool.tile([1, GD], BF16, tag="bvr")
                ones_row = const_pool.tile([1, 512], BF16, tag="ones_row")
                nc.gpsimd.dma_start(bqm[:], bqm_d[:])
                nc.gpsimd.dma_start(bkm[:], bkm_d[:])
                nc.gpsimd.dma_start(bvr[:], bv_d[:])
                nc.gpsimd.dma_start(ones_row[:], ones_row_d[:])

            # PSUM budget (8 banks): score strip-pairs 2x3 banks; the
            # remaining 2 banks are one shared pool whose [128,512] slots
            # serve both projection-chain accumulators and the paired PV
            # outputs (cols 0:65 even qb, 65:130 odd qb).
            with (
                tc.tile_pool(name="ppsum", bufs=2, space="PSUM") as ppsum,
                tc.tile_pool(name="spsum", bufs=2, space="PSUM") as spsum,
                tc.tile_pool(name="att_sb", bufs=10) as att_sb,
                tc.tile_pool(name="fin_sb", bufs=4) as fin_sb,
            ):
                # PE warmup: dependency-free dummy matmuls that run during
                # the startup DMA wait so HAM reaches full clock before the
                # projections start.
                def warm(count):
                    for _ in range(count):
                        wps = ppsum.tile([128, 512], F32, tag="pp")
                        nc.tensor.matmul(
                            wps[:],
                            scratch[:, 0:128],
                            scratch[:],
                            start=True,
                            stop=True,
                            skip_group_check=True,
                        )

                warm(22)

                def qk_chain(m, n, w):
                    wt, dst = (wq, qt) if w == "q" else (wk, kt)
                    ps = ppsum.tile([128, 512], F32, tag="pp")
                    for c in range(KCH):
                        nc.tensor.matmul(
                            ps[:],
                            wt[:, m, c, :],
                            xt[:, 2 * n : 2 * n + 2, c, :],
                            start=(c == 0),
                            stop=(c == KCH - 1 and not has_bias),
                        )
                    if has_bias:
                        br = bqm if w == "q" else bkm
                        nc.tensor.matmul(
                            ps[:],
                            br[:, m, :],
                            ones_row[:],
                            start=False,
                            stop=True,
                        )
                    nc.scalar.activation(
                        dst[:, m, n * 512 : (n + 1) * 512],
                        ps[:],
                        mybir.ActivationFunctionType.Copy,
                    )

                def v_chain(t):
                    ps = ppsum.tile([128, 512], F32, tag="pp")
                    a = (t % 2) * 128
                    for c in range(KCH):
                        nc.tensor.matmul(
                            ps[:],
                            xt[:, t // 2, c, a : a + 128],
                            wv[:, c, :],
                            start=(c == 0),
                            stop=(c == KCH - 1 and not has_bias),
                        )
                    if has_bias:
                        nc.tensor.matmul(
                            ps[:], ones_row[:, :128], bvr[:], start=False, stop=True
                        )
                    nc.scalar.activation(
                        vv[:, t, :, 0:HD], ps[:], mybir.ActivationFunctionType.Copy
                    )

                # QK projections for m-tile 0 (heads 0,1) up front, in x
                # chunk-arrival order (n0 on SP queue, n2 on ACT queue
                # land first); extra warmups bridge the n1/n3 arrival so
                # the PE p-state stays hot. The rest is paced through the
                # attention loops.
                for n in (0, 2, 1, 3):
                    for w in ("q", "k"):
                        qk_chain(0, n, w)

                filler = [lambda t=t: v_chain(t) for t in range(NQB)]
                for m in (1, 2, 3):
                    for n in range(4):
                        for w in ("q", "k"):
                            filler.append(lambda m=m, n=n, w=w: qk_chain(m, n, w))
                fi = 0

                def fill(h, j):
                    # j is a PAIR iteration (8 per head): V chains paced 2
                    # per pair-iter through head 0 (just ahead of the PV
                    # consumers), Wq/Wk m-tiles 1-3 through heads 1-5.
                    nonlocal fi
                    if j >= 8:
                        return
                    take = 0
                    if h == 0:
                        take = 2
                    elif h == 1:
                        take = 1
                    elif h in (2, 3, 4, 5):
                        take = 1 if j % 2 == 0 else 0
                    for _ in range(take):
                        if fi < len(filler):
                            filler[fi]()
                            fi += 1

                # ---- attention: key-block pairs. Each pair-iter scores
                # kb=2j,2j+1 into one [128,2,640] psum strip-pair, exps
                # and masks both with ONE ACT / ONE DVE op (halving their
                # fixed costs), then runs the PV chains for the query-
                # block pair one pair behind (software pipeline). ----
                NPAIR = NQB // 2
                PVLAG = 2  # PV pairs lag score pairs by this many
                for h in range(HPC):
                    mh, p0 = h // 2, (h % 2) * 64
                    strips = {}  # kb -> (ptpair tile, sub idx, {qb: col})
                    for j in range(NPAIR + PVLAG):
                        if j < NPAIR:
                            kbs = (2 * j, 2 * j + 1)
                            ncs = []
                            pspair = spsum.tile([128, 2, 640], F32, tag="sp")
                            ptpair = att_sb.tile([128, 2, 640], BF16, tag="pt")
                            for ii, kb in enumerate(kbs):
                                cons = _consumers(kb)
                                ncols = cons[-1][1] + 128
                                ncs.append(ncols)
                                ps = pspair[:, ii]
                                kslice = kt[
                                    p0 : p0 + 64, mh, kb * 128 : (kb + 1) * 128
                                ]
                                nd = 256 if kb + 1 < NQB else 128
                                nc.tensor.matmul(
                                    ps[:, 0:nd],
                                    kslice,
                                    qt[p0 : p0 + 64, mh, kb * 128 : kb * 128 + nd],
                                    start=True,
                                    stop=(ncols == nd),
                                    skip_group_check=True,
                                )
                                for qb2, col in cons:
                                    if col < 256:
                                        continue
                                    nc.tensor.matmul(
                                        ps[:, col : col + 128],
                                        kslice,
                                        qt[
                                            p0 : p0 + 64,
                                            mh,
                                            qb2 * 128 : (qb2 + 1) * 128,
                                        ],
                                        start=True,
                                        stop=(col + 128 == ncols),
                                        skip_group_check=True,
                                    )
                                strips[kb] = (ptpair, ii, dict(cons))
                            if ncs[0] == ncs[1]:
                                ncm = ncs[0]
                                nc.scalar.activation(
                                    ptpair[:, :, 0:ncm],
                                    pspair[:, :, 0:ncm],
                                    mybir.ActivationFunctionType.Exp,
                                    scale=0.125,
                                )
                                mk = (
                                    masks[:, 2 * j : 2 * j + 2, 0:ncm]
                                    if has_am
                                    else masks[:, :, 0:ncm]
                                )
                                nc.vector.tensor_mul(
                                    ptpair[:, :, 0:ncm], ptpair[:, :, 0:ncm], mk
                                )
                            else:
                                # ragged tail pair (kb 14/15): per-strip ops
                                for ii, kb in enumerate(kbs):
                                    ncols = ncs[ii]
                                    nc.scalar.activation(
                                        ptpair[:, ii, 0:ncols],
                                        pspair[:, ii, 0:ncols],
                                        mybir.ActivationFunctionType.Exp,
                                        scale=0.125,
                                    )
                                    mk = (
                                        masks[:, kb, 0:ncols]
                                        if has_am
                                        else masks[:, ii, 0:ncols]
                                    )
                                    nc.vector.tensor_mul(
                                        ptpair[:, ii, 0:ncols],
                                        ptpair[:, ii, 0:ncols],
                                        mk,
                                    )
                        fill(h, j)
                        if j < PVLAG:
                            continue
                        po = ppsum.tile([128, 512], F32, tag="pp")
                        for qi2 in range(2):
                            qb = 2 * (j - PVLAG) + qi2
                            srcs = [
                                (k, strips[k][0], strips[k][1], strips[k][2][qb])
                                for k in range(max(0, qb - 8), qb + 1)
                                if qb in strips[k][2]
                            ]
                            pslot = po[:, 65 * qi2 : 65 * qi2 + 65]
                            for i, (k, spt, ii, col) in enumerate(srcs):
                                nc.tensor.matmul(
                                    pslot,
                                    spt[:, ii, col : col + 128],
                                    vv[:, k, h, :],
                                    start=(i == 0),
                                    stop=(i == len(srcs) - 1),
                                    skip_group_check=True,
                                )
                        for qi2 in range(2):
                            qb = 2 * (j - PVLAG) + qi2
                            rinv = fin_sb.tile([128, 1], F32, tag="rinv")
                            nc.vector.reciprocal(
                                rinv[:], po[:, 65 * qi2 + HD : 65 * qi2 + HD + 1]
                            )
                            nc.vector.tensor_scalar_mul(
                                outs[:, qb, h * HD : (h + 1) * HD],
                                po[:, 65 * qi2 : 65 * qi2 + HD],
                                rinv[:],
                            )
                        if h == HPC - 1:
                            # alternate output writes between the SP HWDGE
                            # queue and the (idle by now) GpSimd SWDGE
                            # queue so the output stream is not the tail
                            for qi2 in range(2):
                                qb = 2 * (j - PVLAG) + qi2
                                eng = nc.sync if qi2 == 0 else nc.gpsimd
                                eng.dma_start(
                                    out_d[qb * 128 : (qb + 1) * 128, :],
                                    outs[:, qb, :],
                                )
    nc.compile()
    return nc


_CACHE = {}


def _get_program(has_bias, has_am):
    key = (has_bias, has_am)
    if key not in _CACHE:
        _CACHE[key] = build_program(has_bias, has_am)
    return _CACHE[key]


def _host_masks_small():
    """Shift-invariant multiplicative mask strip, replicated twice for
    the paired kb processing: [128, 2, 640] (bf16). Cols 0:128 -> qb=kb
    (delta 0), 128:256 -> qb=kb+1, 256+128i -> far diagonals. Entry = 1
    if allowed else 0. Valid for every kb because the logsparse pattern
    only depends on qi - kj."""
    pi = np.arange(128)[None, :]
    pj = np.arange(128)[:, None]
    m = np.zeros((128, 640), dtype=np.float32)
    m[:, 0:128] = _allowed(pi - pj)
    m[:, 128:256] = _allowed(128 + pi - pj)
    diag = (pi == pj).astype(np.float32)
    for i in range(3):
        m[:, 256 + 128 * i : 384 + 128 * i] = diag
    return np.broadcast_to(m[:, None, :], (128, 2, 640)).copy()


def _host_masks_am(attention_mask_b):
    """Per-kb multiplicative mask strips [128, NQB, 640] with the
    additive attention mask folded in as exp(am[j])."""
    pi = np.arange(128)[None, :]
    pj = np.arange(128)[:, None]
    pat = {}
    for dlt in (0, 1):
        pat[dlt] = _allowed(dlt * 128 + pi - pj).astype(np.float32)
    diag = (pi == pj).astype(np.float32)
    eam = np.exp(attention_mask_b.astype(np.float32))  # [S]
    m = np.zeros((128, NQB, 640), dtype=np.float32)
    for kb in range(NQB):
        amw = eam[kb * 128 : (kb + 1) * 128][:, None]  # [pj, 1]
        m[:, kb, 0:128] = pat[0] * amw
        if kb + 1 < NQB:
            m[:, kb, 128:256] = pat[1] * amw
        for i, d in enumerate(FAR):
            if kb + d < NQB:
                m[:, kb, 256 + 128 * i : 384 + 128 * i] = diag * amw
    return m


def _build_in_maps(
    hidden_states, attention_mask, Wq, bq, Wk, bk, Wv, bv, has_bias, has_am
):
    masks_small = None if has_am else _host_masks_small().astype(NPBF16)
    in_maps = []
    for c in range(8):
        b, g = c // 2, c % 2
        sl = slice(g * GD, (g + 1) * GD)
        im = {
            # [h, s] host-transposed, laid out [h%128, s//256, h//128, s%256]
            "x": np.ascontiguousarray(
                hidden_states[b]
                .T.reshape(KCH, 128, 8, 256)
                .transpose(1, 2, 0, 3)
            ).astype(NPBF16),
            # [c%128, m, c//128, dh%128] m-major column slices
            "wq": np.ascontiguousarray(
                Wq[sl, :].T.reshape(KCH, 128, 4, 128).transpose(1, 2, 0, 3)
            ).astype(NPBF16),
            "wk": np.ascontiguousarray(
                Wk[sl, :].T.reshape(KCH, 128, 4, 128).transpose(1, 2, 0, 3)
            ).astype(NPBF16),
            "wv": np.ascontiguousarray(
                Wv[sl, :].T.reshape(KCH, 128, GD).transpose(1, 0, 2)
            ).astype(NPBF16),
            "masks": (
                _host_masks_am(attention_mask[b, 0, 0, :]).astype(NPBF16)
                if has_am
                else masks_small
            ),
        }
        if has_bias:
            im["bqm"] = bq[sl].reshape(1, 4, 128).astype(NPBF16)
            im["bkm"] = bk[sl].reshape(1, 4, 128).astype(NPBF16)
            im["bv"] = bv[sl].reshape(1, GD).astype(NPBF16)
            im["ones_row"] = np.ones((1, 512), dtype=NPBF16)
        in_maps.append(im)
    return in_maps


def kernel(hidden_states, attention_mask, Wq, bq, Wk, bk, Wv, bv, _trace=False):
    hidden_states = np.asarray(hidden_states)
    attention_mask = np.asarray(attention_mask)
    Wq, bq = np.asarray(Wq), np.asarray(bq)
    Wk, bk = np.asarray(Wk), np.asarray(bk)
    Wv, bv = np.asarray(Wv), np.asarray(bv)

    has_bias = bool(np.any(bq) or np.any(bk) or np.any(bv))
    has_am = bool(np.any(attention_mask))
    nc = _get_program(has_bias, has_am)
    in_maps = _build_in_maps(
        hidden_states, attention_mask, Wq, bq, Wk, bk, Wv, bv, has_bias, has_am
    )

    kw = {}
    if _trace:
        import os
        import shutil

        shutil.rmtree("/tmp/bass_trace", ignore_errors=True)
        os.makedirs("/tmp/bass_trace", exist_ok=True)
        kw = dict(tmpdir="/tmp/bass_trace")
    res = run_bass_kernel_spmd(nc, in_maps, list(range(8)), trace=_trace, **kw)
    out = np.empty((B, S, H), dtype=np.float32)
    for c in range(8):
        b, g = c // 2, c % 2
        out[b, :, g * GD : (g + 1) * GD] = res.results[c]["out"].astype(np.float32)
    if _trace:
        return out, res
    return out


# revision 41
# speedup vs baseline: 1.0197x; 1.0197x over previous
"""LogSparse attention kernel for 8 TRN2 NeuronCores.

Problem: B=4, S=2048, H=1024, 16 heads x 64 dim. Logsparse mask: query i
attends key j iff i-j == 0 or i-j == 2^k (so <=12 keys per query, at
power-of-2 offsets).

Sharding: core c -> batch b = c//2, head-group g = c%2 (8 heads each).
Each core computes q/k/v projections for its (batch, head-group) and the
sparse attention, writing out[b, :, g*512:(g+1)*512].

Device algorithm (per core), v2 (interleaved):
  - DMA-transpose X (bf16) -> XT [h, s] on SBUF in 4 s-chunks so the
    QK projections start as chunks land.
  - QT/KT = W @ XT ([dh, s], dh on partitions), V = X @ WvT ([s, dh]).
  - Attention is key-block-major: key block kb serves query blocks
    qb' in {kb, kb+1} (dense logsparse band, one N=256 matmul) and
    qb' in {kb+2, kb+4, kb+8} (single surviving diagonal each, N=128),
    all sharing the KT[kb] stationary. Scores land in one psum strip
    [128 kj, <=640 qi]; one ACT exp(0.125*s) -> bf16 and one DVE
    multiplicative-mask produce PT. PV (lhsT=PT slice, rhs=V[kb]) and
    row-sum (rhs=ones col) accumulate into per-qb psum; finalize reads
    psum directly (DVE reciprocal + tensor_scalar_mul -> bf16 outs).
  - The attention phase for head h is SOFTWARE-PIPELINED (PV lags
    scores by D=2 key blocks) and INTERLEAVED with the remaining
    projection matmul chains (V during head 0, QK m-tiles 1..3 paced
    through heads 1..5) so the scalar/vector engines' exp/mask work
    hides under projection matmuls instead of pacing the tensor queue.
  - The logsparse mask pattern is shift-invariant across key blocks, so
    when the additive attention_mask is all-zero (the common case) a
    single [128, 640] mask tile is loaded and shared by every kb.
Softmax max-subtraction is skipped: scores*0.125 has std ~0.4 for this
problem family, far from exp overflow. Output is written bf16 (inputs
are unit-scale; bf16 rounding is ~0.4% vs the 2e-2 tolerance) and
upcast to f32 on host.
"""

import numpy as np
import ml_dtypes

import concourse.bass as bass
from concourse import bacc
import concourse.mybir as mybir
from concourse.tile import TileContext
from concourse.bass_utils import run_bass_kernel_spmd

B, S, H = 4, 2048, 1024
NH, HD = 16, 64
G = 2  # head groups per batch
HPC = NH // G  # heads per core = 8
GD = HPC * HD  # 512 group dim
NQB = S // 128  # 16 query blocks
KCH = H // 128  # 8 contraction chunks

BF16 = mybir.dt.bfloat16
F32 = mybir.dt.float32
NPBF16 = ml_dtypes.bfloat16

FAR = (2, 4, 8)
DELAY = 2  # PV lags scores by this many key blocks


def _allowed(diff):
    return (diff == 0) | ((diff > 0) & ((diff & (diff - 1)) == 0))


def _consumers(kb):
    """Query blocks served by key block kb, as (qb, col offset in the
    psum strip). Dense pair first (contiguous N), then far diagonals.
    Valid consumers always form a contiguous col prefix."""
    out = [(kb, 0)]
    if kb + 1 < NQB:
        out.append((kb + 1, 128))
    for i, d in enumerate(FAR):
        if kb + d < NQB:
            out.append((kb + d, 256 + 128 * i))
    return out


def build_program(has_bias: bool, has_am: bool):
    nc = bacc.Bacc("TRN2", target_bir_lowering=False)

    # x arrives pre-transposed from the host ([h%128, h//128, s]) so the
    # load is plain big-packet DMA instead of the slow 256B-packet xbar
    # transpose. Wq/Wk are m-major so each 128-wide dh tile is one small
    # contiguous load (the m=0 slices gate the critical path).
    x_d = nc.declare_dram_parameter("x", [128, 8, KCH, 256], BF16, isOutput=False)
    wq_d = nc.declare_dram_parameter("wq", [128, 4, KCH, 128], BF16, isOutput=False)
    wk_d = nc.declare_dram_parameter("wk", [128, 4, KCH, 128], BF16, isOutput=False)
    wv_d = nc.declare_dram_parameter("wv", [128, KCH, GD], BF16, isOutput=False)
    # multiplicative masks: shift-invariant [pj, 640] strip unless a
    # nonzero additive attention mask forces a per-kb version.
    if has_am:
        masks_d = nc.declare_dram_parameter(
            "masks", [128, NQB, 640], BF16, isOutput=False
        )
    else:
        masks_d = nc.declare_dram_parameter(
            "masks", [128, 2, 640], BF16, isOutput=False
        )
    if has_bias:
        bqm_d = nc.declare_dram_parameter("bqm", [1, 4, 128], BF16, isOutput=False)
        bkm_d = nc.declare_dram_parameter("bkm", [1, 4, 128], BF16, isOutput=False)
        bv_d = nc.declare_dram_parameter("bv", [1, GD], BF16, isOutput=False)
        ones_row_d = nc.declare_dram_parameter(
            "ones_row", [1, 512], BF16, isOutput=False
        )
    out_d = nc.declare_dram_parameter("out", [S, GD], BF16, isOutput=True)

    with TileContext(nc) as tc:
        with (
            tc.tile_pool(name="const", bufs=1) as const_pool,
            tc.tile_pool(name="big", bufs=1) as big_pool,
        ):
            # ---- resident SBUF tensors ----
            # chunk-major so each 512KB s-chunk is one contiguous DMA with
            # 4KB-per-partition descriptors on both the DRAM and SBUF side
            xt = big_pool.tile([128, 8, KCH, 256], BF16, tag="xt")
            wq = big_pool.tile([128, 4, KCH, 128], BF16, tag="wq")
            wk = big_pool.tile([128, 4, KCH, 128], BF16, tag="wk")
            wv = big_pool.tile([128, KCH, GD], BF16, tag="wv")
            qt = big_pool.tile([128, 4, S], BF16, tag="qt")  # [dh%128, m, s]
            kt = big_pool.tile([128, 4, S], BF16, tag="kt")
            vv = big_pool.tile([128, NQB, HPC, HD + 1], BF16, tag="v")
            if has_am:
                masks = const_pool.tile([128, NQB, 640], BF16, tag="masks")
            else:
                masks = const_pool.tile([128, 2, 640], BF16, tag="masks")
            outs = big_pool.tile([128, NQB, GD], BF16, tag="outs")

            # ---- loads on 3 parallel DMA channels (~100 GB/s each):
            # SP HWDGE: x s-chunks 0,1; ACT HWDGE: x s-chunks 2,3;
            # GpSimd SWDGE: m-sliced Wq/Wk (m=0 first, gating the first
            # projections), then Wv + masks + remaining m-slices. ----
            # scratch for PE warmup: memset on gpsimd BEFORE the DMA
            # issues so the first warmup matmul fires ~3us in and the
            # PE clock ramps during the load wait
            scratch = const_pool.tile([128, 512], BF16, tag="warm")
            nc.gpsimd.memset(scratch[:], 0.0)
            for ch in range(4):
                nc.sync.dma_start(xt[:, ch], x_d[:, ch])
            for ch in range(4, 8):
                nc.scalar.dma_start(xt[:, ch], x_d[:, ch])
            nc.scalar.dma_start(wv[:], wv_d[:])
            nc.gpsimd.dma_start(wq[:, 0], wq_d[:, 0])
            nc.gpsimd.dma_start(wk[:, 0], wk_d[:, 0])
            nc.gpsimd.dma_start(masks[:], masks_d[:])
            for m in (1, 2, 3):
                nc.gpsimd.dma_start(wq[:, m], wq_d[:, m])
                nc.gpsimd.dma_start(wk[:, m], wk_d[:, m])
            nc.vector.memset(vv[:, :, :, HD : HD + 1], 1.0)
            if has_bias:
                bqm = const_pool.tile([1, 4, 128], BF16, tag="bqm")
                bkm = const_pool.tile([1, 4, 128], BF16, tag="bkm")
                bvr = const_pool.tile([1, GD], BF16, tag="bvr")
                ones_row = const_pool.tile([1, 512], BF16, tag="ones_row")
                nc.gpsimd.dma_start(bqm[:], bqm_d[:])
                nc.gpsimd.dma_start(bkm[:], bkm_d[:])
                nc.gpsimd.dma_start(bvr[:], bv_d[:])
                nc.gpsimd.dma_start(ones_row[:], ones_row_d[:])

            # PSUM budget (8 banks): score strip-pairs 2x3 banks; the
            # remaining 2 banks are one shared pool whose [128,512] slots
            # serve both projection-chain accumulators and the paired PV
            # outputs (cols 0:65 even qb, 65:130 odd qb).
            with (
                tc.tile_pool(name="ppsum", bufs=2, space="PSUM") as ppsum,
                tc.tile_pool(name="spsum", bufs=2, space="PSUM") as spsum,
                tc.tile_pool(name="att_sb", bufs=10) as att_sb,
                tc.tile_pool(name="fin_sb", bufs=4) as fin_sb,
            ):
                # PE warmup: dependency-free dummy matmuls that run during
                # the startup DMA wait so HAM reaches full clock before the
                # projections start.
                def warm(count):
                    for _ in range(count):
                        wps = ppsum.tile([128, 512], F32, tag="pp")
                        nc.tensor.matmul(
                            wps[:],
                            scratch[:, 0:128],
                            scratch[:],
                            start=True,
                            stop=True,
                            skip_group_check=True,
                        )

                warm(22)

                def qk_chain(m, n, w):
                    wt, dst = (wq, qt) if w == "q" else (wk, kt)
                    ps = ppsum.tile([128, 512], F32, tag="pp")
                    for c in range(KCH):
                        nc.tensor.matmul(
                            ps[:],
                            wt[:, m, c, :],
                            xt[:, 2 * n : 2 * n + 2, c, :],
                            start=(c == 0),
                            stop=(c == KCH - 1 and not has_bias),
                        )
                    if has_bias:
                        br = bqm if w == "q" else bkm
                        nc.tensor.matmul(
                            ps[:],
                            br[:, m, :],
                            ones_row[:],
                            start=False,
                            stop=True,
                        )
                    nc.scalar.activation(
                        dst[:, m, n * 512 : (n + 1) * 512],
                        ps[:],
                        mybir.ActivationFunctionType.Copy,
                    )

                def v_chain(t):
                    ps = ppsum.tile([128, 512], F32, tag="pp")
                    a = (t % 2) * 128
                    for c in range(KCH):
                        nc.tensor.matmul(
                            ps[:],
                            xt[:, t // 2, c, a : a + 128],
                            wv[:, c, :],
                            start=(c == 0),
                            stop=(c == KCH - 1 and not has_bias),
                        )
                    if has_bias:
                        nc.tensor.matmul(
                            ps[:], ones_row[:, :128], bvr[:], start=False, stop=True
                        )
                    nc.scalar.activation(
                        vv[:, t, :, 0:HD], ps[:], mybir.ActivationFunctionType.Copy
                    )

                # QK projections for m-tile 0 (heads 0,1) up front, in x
                # chunk-arrival order (n0 on SP queue, n2 on ACT queue
                # land first); extra warmups bridge the n1/n3 arrival so
                # the PE p-state stays hot. The rest is paced through the
                # attention loops.
                for n in (0, 2, 1, 3):
                    for w in ("q", "k"):
                        qk_chain(0, n, w)

                filler = [lambda t=t: v_chain(t) for t in range(NQB)]
                for m in (1, 2, 3):
                    for n in range(4):
                        for w in ("q", "k"):
                            filler.append(lambda m=m, n=n, w=w: qk_chain(m, n, w))
                fi = 0

                def fill(h, j):
                    # j is a PAIR iteration (8 per head): V chains paced 2
                    # per pair-iter through head 0 (just ahead of the PV
                    # consumers), Wq/Wk m-tiles 1-3 through heads 1-5.
                    nonlocal fi
                    if j >= 8:
                        return
                    take = 0
                    if h == 0:
                        take = 2
                    elif h == 1:
                        take = 1
                    elif h in (2, 3, 4, 5):
                        take = 1 if j % 2 == 0 else 0
                    for _ in range(take):
                        if fi < len(filler):
                            filler[fi]()
                            fi += 1

                # ---- attention: key-block pairs. Each pair-iter scores
                # kb=2j,2j+1 into one [128,2,640] psum strip-pair, exps
                # and masks both with ONE ACT / ONE DVE op (halving their
                # fixed costs), then runs the PV chains for the query-
                # block pair one pair behind (software pipeline). ----
                NPAIR = NQB // 2
                PVLAG = 2  # PV pairs lag score pairs by this many
                for h in range(HPC):
                    mh, p0 = h // 2, (h % 2) * 64
                    strips = {}  # kb -> (ptpair tile, sub idx, {qb: col})
                    for j in range(NPAIR + PVLAG):
                        if j < NPAIR:
                            kbs = (2 * j, 2 * j + 1)
                            ncs = []
                            pspair = spsum.tile([128, 2, 640], F32, tag="sp")
                            ptpair = att_sb.tile([128, 2, 640], BF16, tag="pt")
                            for ii, kb in enumerate(kbs):
                                cons = _consumers(kb)
                                ncols = cons[-1][1] + 128
                                ncs.append(ncols)
                                ps = pspair[:, ii]
                                kslice = kt[
                                    p0 : p0 + 64, mh, kb * 128 : (kb + 1) * 128
                                ]
                                nd = 256 if kb + 1 < NQB else 128
                                nc.tensor.matmul(
                                    ps[:, 0:nd],
                                    kslice,
                                    qt[p0 : p0 + 64, mh, kb * 128 : kb * 128 + nd],
                                    start=True,
                                    stop=(ncols == nd),
                                    skip_group_check=True,
                                )
                                for qb2, col in cons:
                                    if col < 256:
                                        continue
                                    nc.tensor.matmul(
                                        ps[:, col : col + 128],
                                        kslice,
                                        qt[
                                            p0 : p0 + 64,
                                            mh,
                                            qb2 * 128 : (qb2 + 1) * 128,
                                        ],
                                        start=True,
                                        stop=(col + 128 == ncols),
                                        skip_group_check=True,
                                    )
                                strips[kb] = (ptpair, ii, dict(cons))
                            if ncs[0] == ncs[1]:
                                ncm = ncs[0]
                                nc.scalar.activation(
                                    ptpair[:, :, 0:ncm],
                                    pspair[:, :, 0:ncm],
                                    mybir.ActivationFunctionType.Exp,
                                    scale=0.125,
                                )
                                mk = (
                                    masks[:, 2 * j : 2 * j + 2, 0:ncm]
                                    if has_am
                                    else masks[:, :, 0:ncm]
                                )
                                nc.vector.tensor_mul(
                                    ptpair[:, :, 0:ncm], ptpair[:, :, 0:ncm], mk
                                )
                            else:
                                # ragged tail pair (kb 14/15): per-strip ops
                                for ii, kb in enumerate(kbs):
                                    ncols = ncs[ii]
                                    nc.scalar.activation(
                                        ptpair[:, ii, 0:ncols],
                                        pspair[:, ii, 0:ncols],
                                        mybir.ActivationFunctionType.Exp,
                                        scale=0.125,
                                    )
                                    mk = (
                                        masks[:, kb, 0:ncols]
                                        if has_am
                                        else masks[:, ii, 0:ncols]
                                    )
                                    nc.vector.tensor_mul(
                                        ptpair[:, ii, 0:ncols],
                                        ptpair[:, ii, 0:ncols],
                                        mk,
                                    )
                        fill(h, j)
                        if j < PVLAG:
                            continue
                        po = ppsum.tile([128, 512], F32, tag="pp")
                        for qi2 in range(2):
                            qb = 2 * (j - PVLAG) + qi2
                            srcs = [
                                (k, strips[k][0], strips[k][1], strips[k][2][qb])
                                for k in range(max(0, qb - 8), qb + 1)
                                if qb in strips[k][2]
                            ]
                            pslot = po[:, 65 * qi2 : 65 * qi2 + 65]
                            for i, (k, spt, ii, col) in enumerate(srcs):
                                nc.tensor.matmul(
                                    pslot,
                                    spt[:, ii, col : col + 128],
                                    vv[:, k, h, :],
                                    start=(i == 0),
                                    stop=(i == len(srcs) - 1),
                                    skip_group_check=True,
                                )
                        for qi2 in range(2):
                            qb = 2 * (j - PVLAG) + qi2
                            rinv = fin_sb.tile([128, 1], F32, tag="rinv")
                            nc.vector.reciprocal(
                                rinv[:], po[:, 65 * qi2 + HD : 65 * qi2 + HD + 1]
                            )
                            nc.vector.tensor_scalar_mul(
                                outs[:, qb, h * HD : (h + 1) * HD],
                                po[:, 65 * qi2 : 65 * qi2 + HD],
                                rinv[:],
                            )
                        if h == HPC - 1:
                            # alternate output writes between the SP HWDGE
                            # queue and the (idle by now) GpSimd SWDGE
                            # queue so the output stream is not the tail
                            for qi2 in range(2):
                                qb = 2 * (j - PVLAG) + qi2
                                eng = nc.sync if qi2 == 0 else nc.gpsimd
                                eng.dma_start(
                                    out_d[qb * 128 : (qb + 1) * 128, :],
                                    outs[:, qb, :],
                                )
    nc.compile()
    return nc


_CACHE = {}


def _get_program(has_bias, has_am):
    key = (has_bias, has_am)
    if key not in _CACHE:
        _CACHE[key] = build_program(has_bias, has_am)
    return _CACHE[key]


def _host_masks_small():
    """Shift-invariant multiplicative mask strip, replicated twice for
    the paired kb processing: [128, 2, 640] (bf16). Cols 0:128 -> qb=kb
    (delta 0), 128:256 -> qb=kb+1, 256+128i -> far diagonals. Entry = 1
    if allowed else 0. Valid for every kb because the logsparse pattern
    only depends on qi - kj."""
    pi = np.arange(128)[None, :]
    pj = np.arange(128)[:, None]
    m = np.zeros((128, 640), dtype=np.float32)
    m[:, 0:128] = _allowed(pi - pj)
    m[:, 128:256] = _allowed(128 + pi - pj)
    diag = (pi == pj).astype(np.float32)
    for i in range(3):
        m[:, 256 + 128 * i : 384 + 128 * i] = diag
    return np.broadcast_to(m[:, None, :], (128, 2, 640)).copy()


def _host_masks_am(attention_mask_b):
    """Per-kb multiplicative mask strips [128, NQB, 640] with the
    additive attention mask folded in as exp(am[j])."""
    pi = np.arange(128)[None, :]
    pj = np.arange(128)[:, None]
    pat = {}
    for dlt in (0, 1):
        pat[dlt] = _allowed(dlt * 128 + pi - pj).astype(np.float32)
    diag = (pi == pj).astype(np.float32)
    eam = np.exp(attention_mask_b.astype(np.float32))  # [S]
    m = np.zeros((128, NQB, 640), dtype=np.float32)
    for kb in range(NQB):
        amw = eam[kb * 128 : (kb + 1) * 128][:, None]  # [pj, 1]
        m[:, kb, 0:128] = pat[0] * amw
        if kb + 1 < NQB:
            m[:, kb, 128:256] = pat[1] * amw
        for i, d in enumerate(FAR):
            if kb + d < NQB:
                m[:, kb, 256 + 128 * i : 384 + 128 * i] = diag * amw
    return m


def _build_in_maps(
    hidden_states, attention_mask, Wq, bq, Wk, bk, Wv, bv, has_bias, has_am
):
    masks_small = None if has_am else _host_masks_small().astype(NPBF16)
    in_maps = []
    for c in range(8):
        b, g = c // 2, c % 2
        sl = slice(g * GD, (g + 1) * GD)
        im = {
            # [h, s] host-transposed, laid out [h%128, s//256, h//128, s%256]
            "x": np.ascontiguousarray(
                hidden_states[b]
                .T.reshape(KCH, 128, 8, 256)
                .transpose(1, 2, 0, 3)
            ).astype(NPBF16),
            # [c%128, m, c//128, dh%128] m-major column slices
            "wq": np.ascontiguousarray(
                Wq[sl, :].T.reshape(KCH, 128, 4, 128).transpose(1, 2, 0, 3)
            ).astype(NPBF16),
            "wk": np.ascontiguousarray(
                Wk[sl, :].T.reshape(KCH, 128, 4, 128).transpose(1, 2, 0, 3)
            ).astype(NPBF16),
            "wv": np.ascontiguousarray(
                Wv[sl, :].T.reshape(KCH, 128, GD).transpose(1, 0, 2)
            ).astype(NPBF16),
            "masks": (
                _host_masks_am(attention_mask[b, 0, 0, :]).astype(NPBF16)
                if has_am
                else masks_small
            ),
        }
        if has_bias:
            im["bqm"] = bq[sl].reshape(1, 4, 128).astype(NPBF16)
            im["bkm"] = bk[sl].reshape(1, 4, 128).astype(NPBF16)
            im["bv"] = bv[sl].reshape(1, GD).astype(NPBF16)
            im["ones_row"] = np.ones((1, 512), dtype=NPBF16)
        in_maps.append(im)
    return in_maps


def kernel(hidden_states, attention_mask, Wq, bq, Wk, bk, Wv, bv, _trace=False):
    hidden_states = np.asarray(hidden_states)
    attention_mask = np.asarray(attention_mask)
    Wq, bq = np.asarray(Wq), np.asarray(bq)
    Wk, bk = np.asarray(Wk), np.asarray(bk)
    Wv, bv = np.asarray(Wv), np.asarray(bv)

    has_bias = bool(np.any(bq) or np.any(bk) or np.any(bv))
    has_am = bool(np.any(attention_mask))
    nc = _get_program(has_bias, has_am)
    in_maps = _build_in_maps(
        hidden_states, attention_mask, Wq, bq, Wk, bk, Wv, bv, has_bias, has_am
    )

    kw = {}
    if _trace:
        import os
        import shutil

        shutil.rmtree("/tmp/bass_trace", ignore_errors=True)
        os.makedirs("/tmp/bass_trace", exist_ok=True)
        kw = dict(tmpdir="/tmp/bass_trace")
    res = run_bass_kernel_spmd(nc, in_maps, list(range(8)), trace=_trace, **kw)
    out = np.empty((B, S, H), dtype=np.float32)
    for c in range(8):
        b, g = c // 2, c % 2
        out[b, :, g * GD : (g + 1) * GD] = res.results[c]["out"].astype(np.float32)
    if _trace:
        return out, res
    return out
